# revision 7
# baseline (speedup 1.0000x reference)
"""ComplexGRUCell forward on 8 Trainium2 NeuronCores.

Strategy (data-parallel, feat-major compute):
  - Shard batch B=65536 across 8 cores (8192 rows each).
  - Host-side: transpose x/h slices to [256, 8192] (feature-major) and
    pre-combine the 6 complex weight pairs into 8 stacked real matrices
    (one per pre-activation accumulator), transposed into matmul-stationary
    layout. Biases pre-combined per accumulator.
  - Device: for each 512-column batch tile, accumulate the 8 gate
    pre-activations with fp32r matmuls (features on partitions, batch on the
    free dim), apply sigmoid/tanh with per-partition biases on the scalar
    engine, do the complex-arithmetic elementwise work on the vector engine,
    and DMA the feature-major outputs back.
  - Host-side: transpose outputs back to [B, 256] and stack real/imag.

Self-contained: hardcodes B=65536, I=H=256, 8 cores.
"""

import numpy as np

import concourse.bass as bass  # noqa: F401
import concourse.mybir as mybir
import concourse.tile as tile
from concourse import bacc, bass_utils

F32 = mybir.dt.float32
F32R = mybir.dt.float32r
FP16 = mybir.dt.float16
AF = mybir.ActivationFunctionType

B_TOTAL = 65536
N_CORES = 8
B_LOC = B_TOTAL // N_CORES  # 8192
H = 256
NB = 512                    # batch columns per tile
N_TILES = B_LOC // NB       # 16
KC = H // 128               # 2 feature chunks (partition dim)

_GATE_ACCS = ["r_re", "r_im", "z_re", "z_im"]      # 8 k-chunks each
_CAND_ACCS = ["x3_re", "x3_im", "g3_re", "g3_im"]  # 4 k-chunks each
_STREAMS = ["xrT", "xiT", "hrT", "hiT"]

# Module-level knobs for the test harness (grading path leaves them alone).
TRACE = False
LAST_RESULT = None

_CACHED_NC = None


def _build_nc():
    nc = bacc.Bacc("TRN2", target_bir_lowering=False, debug=False,
                   num_devices=N_CORES)

    ins = {}
    for s in _STREAMS:
        ins[s] = nc.dram_tensor(s, (H, B_LOC), F32R, kind="ExternalInput")
    for g in _GATE_ACCS:
        ins["w_" + g] = nc.dram_tensor("w_" + g, (128, 8 * 256), F32R,
                                       kind="ExternalInput")
    for g in _CAND_ACCS:
        ins["w_" + g] = nc.dram_tensor("w_" + g, (128, 4 * 256), F32R,
                                       kind="ExternalInput")
    ins["biases"] = nc.dram_tensor("biases", (128, 16), F32,
                                   kind="ExternalInput")
    out_r = nc.dram_tensor("outT_r", (H, B_LOC), F32, kind="ExternalOutput")
    out_i = nc.dram_tensor("outT_i", (H, B_LOC), F32, kind="ExternalOutput")

    bias_col = {}
    for gi, g in enumerate(_GATE_ACCS + _CAND_ACCS):
        for mo in range(2):
            bias_col[(g, mo)] = gi * 2 + mo

    with tile.TileContext(nc) as tc:
        with (
            tc.tile_pool(name="wpool", bufs=1) as wpool,
            tc.tile_pool(name="mvpool", bufs=2) as mvpool,
            tc.tile_pool(name="spool", bufs=3) as spool,
            tc.tile_pool(name="tpool", bufs=2) as tpool,
            tc.tile_pool(name="opool", bufs=3) as opool,
            tc.tile_pool(name="psum", bufs=1, space="PSUM") as psum,
        ):
            # ---- one-time weight/bias loads -------------------------------
            # Ordered so only the r-gate weights gate the first matmuls:
            # r weights -> tile-0 data -> remaining weights.
            wt = {}

            def load_w(g, n):
                t = wpool.tile([128, n * 256], F32R, name=f"wt_{g}",
                               tag=f"wt_{g}")
                nc.sync.dma_start(t[:], ins["w_" + g][:])
                wt[g] = t

            def load_mv(c0, nb):
                mv = {}
                for si, s in enumerate(_STREAMS):
                    for k in range(KC):
                        m = mvpool.tile([128, nb], F32R, name=f"mv{si}{k}",
                                        tag=f"mv{si}{k}",
                                        padded_shape=[128, NB])
                        nc.sync.dma_start(
                            m[:], ins[s][k * 128:(k + 1) * 128, c0:c0 + nb])
                        mv[(si, k)] = m
                return mv

            load_w("r_re", 8)
            mv0 = load_mv(0, NB)
            load_w("r_im", 8)
            for g in ("z_re", "z_im"):
                load_w(g, 8)
            for g in _CAND_ACCS:
                load_w(g, 4)
            bt = wpool.tile([128, 16], F32, name="bias_t", tag="bias_t")
            nc.sync.dma_start(bt[:], ins["biases"][:])

            def bias_ap(g, mo):
                c = bias_col[(g, mo)]
                return bt[:, c:c + 1]

            # ---- per batch tile -------------------------------------------
            schedule = [(i * NB, NB) for i in range(N_TILES - 1)]
            half_nb = NB // 2
            last0 = (N_TILES - 1) * NB
            schedule += [(last0, half_nb), (last0 + half_nb, half_nb)]
            for t_idx, (c0, nb) in enumerate(schedule):
                mv = mv0 if t_idx == 0 else load_mv(c0, nb)

                def mk_pair(nm, tag):
                    return psum.tile([128, 2 * nb], F32, name=nm, tag=tag)

                p_r = [mk_pair(f"p_r{mo}", f"bankA{mo}") for mo in range(2)]
                p_z = [mk_pair(f"p_z{mo}", f"bankB{mo}") for mo in range(2)]

                def accum(pair, half, g, mo, streams):
                    n_mm = len(streams) * KC
                    j = 0
                    for si in streams:
                        for k in range(KC):
                            ki = (si - streams[0]) * KC + k
                            w_ap = wt[g][:, ki * 256 + mo * 128:
                                         ki * 256 + (mo + 1) * 128]
                            nc.tensor.matmul(
                                pair[:, half * nb:(half + 1) * nb],
                                w_ap, mv[(si, k)][:],
                                start=(j == 0), stop=(j == n_mm - 1))
                            j += 1

                ALL, XS, HS = [0, 1, 2, 3], [0, 1], [2, 3]
                for mo in range(2):
                    accum(p_r[mo], 0, "r_re", mo, ALL)
                    accum(p_r[mo], 1, "r_im", mo, ALL)
                for mo in range(2):
                    accum(p_z[mo], 0, "z_re", mo, ALL)
                    accum(p_z[mo], 1, "z_im", mo, ALL)

                p_g3 = [mk_pair(f"p_g{mo}", f"bankA{mo}") for mo in range(2)]
                p_x3 = [mk_pair(f"p_x{mo}", f"bankB{mo}") for mo in range(2)]
                for mo in range(2):
                    accum(p_g3[mo], 0, "g3_re", mo, HS)
                    accum(p_g3[mo], 1, "g3_im", mo, HS)
                for mo in range(2):
                    accum(p_x3[mo], 0, "x3_re", mo, XS)
                    accum(p_x3[mo], 1, "x3_im", mo, XS)

                # ---- elementwise epilogue per feature chunk ----------------
                for mo in range(2):
                    sr = spool.tile([128, 2 * nb], F32, name=f"sr{mo}", tag="sr")
                    sz = spool.tile([128, 2 * nb], F32, name=f"sz{mo}", tag="sz")
                    g3 = spool.tile([128, 2 * nb], F32, name=f"g3{mo}", tag="g3")
                    nn = spool.tile([128, 2 * nb], F32, name=f"nn{mo}", tag="nn")

                    nc.scalar.activation(sr[:, 0:nb], p_r[mo][:, 0:nb],
                                         AF.Sigmoid, bias=bias_ap("r_re", mo))
                    nc.scalar.activation(sr[:, nb:], p_r[mo][:, nb:],
                                         AF.Sigmoid, bias=bias_ap("r_im", mo))
                    nc.scalar.activation(sz[:, 0:nb], p_z[mo][:, 0:nb],
                                         AF.Sigmoid, bias=bias_ap("z_re", mo))
                    nc.scalar.activation(sz[:, nb:], p_z[mo][:, nb:],
                                         AF.Sigmoid, bias=bias_ap("z_im", mo))
                    nc.scalar.activation(g3[:, 0:nb], p_g3[mo][:, 0:nb],
                                         AF.Identity, bias=bias_ap("g3_re", mo))
                    nc.scalar.activation(g3[:, nb:], p_g3[mo][:, nb:],
                                         AF.Identity, bias=bias_ap("g3_im", mo))

                    # h3 = r * g3 (complex)
                    u = tpool.tile([128, 2 * nb], F32, name=f"u{mo}", tag="u")
                    v = tpool.tile([128, 2 * nb], F32, name=f"v{mo}", tag="v")
                    h3 = tpool.tile([128, 2 * nb], F32, name=f"h3{mo}", tag="h3")
                    ss = tpool.tile([128, 2 * nb], F32, name=f"ss{mo}", tag="ss")
                    nc.vector.tensor_mul(u[:], sr[:], g3[:])   # rr*g3r | ri*g3i
                    nc.vector.tensor_mul(v[:, 0:nb], sr[:, 0:nb], g3[:, nb:])
                    nc.vector.tensor_mul(v[:, nb:], sr[:, nb:], g3[:, 0:nb])
                    nc.vector.tensor_sub(h3[:, 0:nb], u[:, 0:nb], u[:, nb:])
                    nc.vector.tensor_add(h3[:, nb:], v[:, 0:nb], v[:, nb:])
                    # s = x3 + h3 ; n = tanh(s + bias_x3)
                    nc.vector.tensor_add(ss[:], p_x3[mo][:], h3[:])
                    nc.scalar.activation(nn[:, 0:nb], ss[:, 0:nb],
                                         AF.Tanh, bias=bias_ap("x3_re", mo))
                    nc.scalar.activation(nn[:, nb:], ss[:, nb:],
                                         AF.Tanh, bias=bias_ap("x3_im", mo))

                    # d = h - n ; out = n + z*d (complex)
                    d = tpool.tile([128, 2 * nb], F32, name=f"d{mo}", tag="d")
                    p = tpool.tile([128, 2 * nb], F32, name=f"p{mo}", tag="p")
                    q = tpool.tile([128, 2 * nb], F32, name=f"q{mo}", tag="q")
                    tm = tpool.tile([128, 2 * nb], F32, name=f"tm{mo}", tag="tm")
                    ot = opool.tile([128, 2 * nb], F32, name=f"ot{mo}", tag="ot")
                    nc.vector.tensor_sub(d[:, 0:nb],
                                         mv[(2, mo)][:].bitcast(F32), nn[:, 0:nb])
                    nc.vector.tensor_sub(d[:, nb:],
                                         mv[(3, mo)][:].bitcast(F32), nn[:, nb:])
                    nc.vector.tensor_mul(p[:], sz[:], d[:])    # zr*dr | zi*di
                    nc.vector.tensor_mul(q[:, 0:nb], sz[:, 0:nb], d[:, nb:])
                    nc.vector.tensor_mul(q[:, nb:], sz[:, nb:], d[:, 0:nb])
                    nc.vector.tensor_sub(tm[:, 0:nb], p[:, 0:nb], p[:, nb:])
                    nc.vector.tensor_add(tm[:, nb:], q[:, 0:nb], q[:, nb:])
                    nc.vector.tensor_add(ot[:], nn[:], tm[:])

                    nc.sync.dma_start(
                        out_r[mo * 128:(mo + 1) * 128, c0:c0 + nb], ot[:, 0:nb])
                    nc.sync.dma_start(
                        out_i[mo * 128:(mo + 1) * 128, c0:c0 + nb], ot[:, nb:])

    nc.compile()
    return nc


def _prep_weights(p):
    """Host-side weight/bias combination -> device layouts."""
    def stk(mats):  # list of [256,256] -> stationary layout [128, n*256]
        W = np.concatenate(mats, axis=1)          # [out=256, in_total]
        WT = np.ascontiguousarray(W.T)            # [in_total, 256]
        n = WT.shape[0] // 128
        return np.ascontiguousarray(
            WT.reshape(n, 128, 256).transpose(1, 0, 2).reshape(128, n * 256)
        ).astype(np.float32)

    w = {}
    w["w_r_re"] = stk([p["w1Wr"], -p["w1Wi"], p["r1Wr"], -p["r1Wi"]])
    w["w_r_im"] = stk([p["w1Wi"], p["w1Wr"], p["r1Wi"], p["r1Wr"]])
    w["w_z_re"] = stk([p["w2Wr"], -p["w2Wi"], p["r2Wr"], -p["r2Wi"]])
    w["w_z_im"] = stk([p["w2Wi"], p["w2Wr"], p["r2Wi"], p["r2Wr"]])
    w["w_x3_re"] = stk([p["w3Wr"], -p["w3Wi"]])
    w["w_x3_im"] = stk([p["w3Wi"], p["w3Wr"]])
    w["w_g3_re"] = stk([p["r3Wr"], -p["r3Wi"]])
    w["w_g3_im"] = stk([p["r3Wi"], p["r3Wr"]])

    bias = {
        "r_re": p["w1br"] - p["w1bi"] + p["r1br"] - p["r1bi"],
        "r_im": p["w1br"] + p["w1bi"] + p["r1br"] + p["r1bi"],
        "z_re": p["w2br"] - p["w2bi"] + p["r2br"] - p["r2bi"],
        "z_im": p["w2br"] + p["w2bi"] + p["r2br"] + p["r2bi"],
        "x3_re": p["w3br"] - p["w3bi"],
        "x3_im": p["w3br"] + p["w3bi"],
        "g3_re": p["r3br"] - p["r3bi"],
        "g3_im": p["r3br"] + p["r3bi"],
    }
    bcols = np.zeros((128, 16), dtype=np.float32)
    for gi, g in enumerate(_GATE_ACCS + _CAND_ACCS):
        for mo in range(2):
            bcols[:, gi * 2 + mo] = np.asarray(bias[g])[mo * 128:(mo + 1) * 128]
    w["biases"] = bcols
    return w


def kernel(**inputs):
    global _CACHED_NC, LAST_RESULT
    if _CACHED_NC is None:
        _CACHED_NC = _build_nc()
    nc = _CACHED_NC

    wmaps = _prep_weights(inputs)

    in_maps = []
    for c in range(N_CORES):
        sl = slice(c * B_LOC, (c + 1) * B_LOC)
        m = dict(wmaps)
        m["xrT"] = np.ascontiguousarray(np.asarray(inputs["xr"])[sl].T,
                                        dtype=np.float32)
        m["xiT"] = np.ascontiguousarray(np.asarray(inputs["xi"])[sl].T,
                                        dtype=np.float32)
        m["hrT"] = np.ascontiguousarray(np.asarray(inputs["hr"])[sl].T,
                                        dtype=np.float32)
        m["hiT"] = np.ascontiguousarray(np.asarray(inputs["hi"])[sl].T,
                                        dtype=np.float32)
        in_maps.append(m)

    kwargs = {}
    if TRACE:
        import sys, types
        try:
            from trn_agent_boot.trn_boot import _ntff_profile_via_ctypes
            mod = types.ModuleType("antenv.axon_hooks")
            mod._hook = _ntff_profile_via_ctypes('/opt/axon/libaxon_pjrt.so')
            mod.get_axon_ntff_profile_hook = lambda: mod._hook
            mod.set_axon_ntff_profile_hook = (
                lambda h: setattr(mod, "_hook", h))
            sys.modules["antenv.axon_hooks"] = mod
            kwargs["trace"] = True
        except Exception:
            pass

    res = bass_utils.run_bass_kernel_spmd(
        nc, in_maps, core_ids=list(range(N_CORES)), **kwargs)
    LAST_RESULT = res

    out = np.empty((2, B_TOTAL, H), dtype=np.float32)
    for c in range(N_CORES):
        sl = slice(c * B_LOC, (c + 1) * B_LOC)
        out[0, sl] = res.results[c]["outT_r"].T
        out[1, sl] = res.results[c]["outT_i"].T
    return out
